# revision 1
# baseline (speedup 1.0000x reference)
"""Trainium2 Bass kernel for nn_AutoEncoder3D (chamfer-loss autoencoder).

Strategy (8 NeuronCores, SPMD with per-core data):
  core c -> batch b = c // 4, quarter q = c % 4 of generated points.
  Each core: full encoder (f32 matmuls), decoder for its quarter of the
  3072 output columns (f32 matmuls), then fused cdist+min over the
  [16384, 1024] chamfer block using a lifted-embedding fp16 hi/lo matmul
  (K=20, ~1e-5 exact) on the PE, ACT PSUM->SBUF fp16 copies, and DVE
  tt-min fold chains (2x mode) for row-min and running col-min.
  Host combines per-core row-min sums and col-min partials.
"""

import numpy as np

import concourse.bass as bass
import concourse.mybir as mybir
import concourse.tile as tile_mod
from concourse.bass_utils import run_bass_kernel_spmd
from concourse.masks import make_identity
from concourse.tile import ScopedClock, TileContext

F32 = mybir.dt.float32
F32R = mybir.dt.float32r
F16 = mybir.dt.float16
BF16 = mybir.dt.bfloat16
A = mybir.AluOpType
AFT = mybir.ActivationFunctionType
AX = mybir.AxisListType

B = 2
G = 64
M = 1024
NCORES = 8
JQ = 256          # generated points per grid cell handled per core
NLOC = G * JQ     # 16384 generated points per core
NT = NLOC // 128  # 128 n-tiles
TB = 8            # n-tiles per DVE batch
NB = NT // TB


# ---------------------------------------------------------------------------
# Tile-framework patches: this walrus build allows at most ONE sync wait per
# instruction.  (a) split multi-wait instructions with preceding no-ops,
# (b) replace the context-exit drain (which carries one wait per live proc)
# with individual SP wait_ge instructions.
# ---------------------------------------------------------------------------
if not getattr(tile_mod, "_ae3d_wait_patch", False):
    tile_mod._ae3d_wait_patch = True


    _orig_commit = tile_mod.TileContext._commit_instruction

    def _commit_split(self, inst, lazy_reg_writes=True):
        si = getattr(inst, "sync_info", None)
        if si is not None and si.on_wait and len(si.on_wait) > 1:
            waits = list(si.on_wait)
            for w in waits[:-1]:
                nop = mybir.InstNoOp(
                    name=self.nc.get_next_instruction_name(),
                    sync_info=mybir.SyncInfo(on_wait=[w], on_update=[]),
                    bass_nofuse=True,
                    engine=inst.engine,
                )
                _orig_commit(self, nop, lazy_reg_writes)
            inst.sync_info = mybir.SyncInfo(
                on_wait=[waits[-1]], on_update=list(si.on_update)
            )
        return _orig_commit(self, inst, lazy_reg_writes)

    tile_mod.TileContext._commit_instruction = _commit_split

    def _patched_drain_and_barrier(self, tick_clock, wait_clock):
        gc = tick_clock.global_clock
        alloc = self.sems.allocated()
        for proc, sem in sorted(alloc.items()):
            tick = gc[proc]
            if tick > 0:
                mult = 16 if sem.name.startswith("DMA") else 1
                self.nc.sync.wait_ge(sem, tick * mult)
        self.nc.sync.drain()
        self.nc.all_engine_barrier()
        assert self.sems is not None
        popped = self.nc._tile_sem_poison_stack.pop()
        assert popped is self._sem_poison
        self.nc.clear_and_free_semaphores(list(self.sems.allocated().values()))
        self.nc.all_engine_barrier()

    tile_mod.TileContext._drain_and_barrier = _patched_drain_and_barrier


# ---------------------------------------------------------------------------
# Device program
# ---------------------------------------------------------------------------
def _build_nc():
    nc = bass.Bass()

    xft = nc.dram_tensor("xft", [128, 25], F32R, kind="ExternalInput")
    w1 = nc.dram_tensor("w1", [128, 25, 512], F32R, kind="ExternalInput")
    w2 = nc.dram_tensor("w2", [128, 5, 128], F32R, kind="ExternalInput")
    w3 = nc.dram_tensor("w3", [128, 2, 64], F32R, kind="ExternalInput")
    wd1 = nc.dram_tensor("wd1", [64, 128], F32R, kind="ExternalInput")
    wd1g = nc.dram_tensor("wd1g", [4, 128], F32R, kind="ExternalInput")
    gridt = nc.dram_tensor("gridt", [4, 64], F32R, kind="ExternalInput")
    wd2 = nc.dram_tensor("wd2", [128, 2, 512], F32R, kind="ExternalInput")
    wd3 = nc.dram_tensor("wd3", [128, 5, 768], F32R, kind="ExternalInput")
    s3tl = nc.dram_tensor("s3tl", [128, 8, 3], F32, kind="ExternalInput")
    onespad = nc.dram_tensor("onespad", [128, 1], F32R, kind="ExternalInput")

    colpart = nc.dram_tensor("colpart", [128, 8], F32, kind="ExternalOutput")
    rowsumv = nc.dram_tensor("rowsumv", [128, 1], F32, kind="ExternalOutput")

    dsc = nc.dram_tensor("dsc", [64, 2560], BF16)   # bounce for phiT gather

    with TileContext(nc) as tc:
        with tc.tile_pool(name="pers", bufs=1) as pers, \
             tc.tile_pool(name="wts", bufs=1) as wts, \
             tc.tile_pool(name="ps", bufs=4, space="PSUM") as psp:

            # ---------------- persistent weight DMAs ----------------
            w2t = wts.tile([128, 5, 128], F32R)
            nc.sync.dma_start(w2t[:], w2[:])
            w3t = wts.tile([128, 2, 64], F32R)
            nc.sync.dma_start(w3t[:], w3[:])
            wd1t = wts.tile([64, 128], F32R)
            nc.sync.dma_start(wd1t[:], wd1[:])
            wd1gt = wts.tile([4, 128], F32R)
            nc.sync.dma_start(wd1gt[:], wd1g[:])
            gridtt = wts.tile([4, 64], F32R)
            nc.sync.dma_start(gridtt[:], gridt[:])
            wd2t = wts.tile([128, 2, 512], F32R)
            nc.sync.dma_start(wd2t[:], wd2[:])
            wd3t = wts.tile([128, 5, 768], F32R)
            nc.sync.dma_start(wd3t[:], wd3[:])
            onesp = wts.tile([128, 1], F32R)
            nc.sync.dma_start(onesp[:], onespad[:])
            ident = wts.tile([128, 128], F32)
            make_identity(nc, ident[:])
            identh = wts.tile([128, 128], BF16)
            make_identity(nc, identh[:])

            psiT = pers.tile([20, 1024], BF16)
            phiT = pers.tile([20, 16384], BF16)
            colrun = pers.tile([128, TB, 1024], F16)
            nc.gpsimd.memset(colrun[:], 60000.0)
            rowstore = pers.tile([128, NT], F32)
            h1T = pers.tile([128, 5], F32R)
            h2T = pers.tile([128, 2], F32R)
            zrelu = pers.tile([64, 1], F32)
            zbT = pers.tile([64, 64], F32R)
            onesb = pers.tile([128, 64], F32R)
            h1d = pers.tile([64, 128], F32)
            h1dT = pers.tile([128, 64], F32R)
            h2d = pers.tile([64, 512], F32)
            h2dT = pers.tile([128, 4, 64], F32R)
            c1f = pers.tile([128, 1024], F32)
            colpartT = pers.tile([128, 8], F32)
            rsv = pers.tile([128, 1], F32)

            with tc.tile_pool(name="tmp", bufs=1) as tmp:
                # ---------------- psi (target lift) ----------------
                # stage k-layout: [h(5), h(5), l(5), l(5)]; m = mt*128 + p
                s3t = tmp.tile([128, 8, 3], F32)
                nc.sync.dma_start(s3t[:], s3tl[:])
                stage = tmp.tile([128, 8, 20], BF16)
                sq = tmp.tile([128, 8, 3], F32)
                nc.vector.tensor_tensor(sq[:], s3t[:], s3t[:], op=A.mult)
                s2t = tmp.tile([128, 8], F32)
                nc.vector.tensor_reduce(s2t[:], sq[:], axis=AX.X, op=A.add)
                m2 = tmp.tile([128, 8, 3], F32)
                nc.vector.tensor_scalar_mul(m2[:], s3t[:], -2.0)
                s2v = s2t[:].rearrange("p (t o) -> p t o", o=1)
                nc.vector.tensor_copy(stage[:, :, 0:3], m2[:])
                nc.vector.tensor_copy(stage[:, :, 5:8], stage[:, :, 0:3])
                nc.vector.memset(stage[:, :, 3:4], 1.0)
                nc.vector.memset(stage[:, :, 8:9], 1.0)
                nc.vector.tensor_copy(stage[:, :, 4:5], s2v)
                nc.vector.tensor_copy(stage[:, :, 9:10], stage[:, :, 4:5])
                m2hf = tmp.tile([128, 8, 3], F32)
                nc.vector.tensor_copy(m2hf[:], stage[:, :, 0:3])
                nc.vector.tensor_tensor(
                    stage[:, :, 10:13], m2[:], m2hf[:], op=A.subtract
                )
                nc.vector.tensor_copy(stage[:, :, 15:18], stage[:, :, 10:13])
                nc.vector.memset(stage[:, :, 13:14], 0.0)
                nc.vector.memset(stage[:, :, 18:19], 0.0)
                s2hf = tmp.tile([128, 8], F32)
                nc.vector.tensor_copy(s2hf[:], stage[:, :, 4:5])
                nc.vector.tensor_tensor(
                    stage[:, :, 14:15], s2v,
                    s2hf[:].rearrange("p (t o) -> p t o", o=1), op=A.subtract,
                )
                nc.vector.tensor_copy(stage[:, :, 19:20], stage[:, :, 14:15])
                for mt in range(8):
                    psm = psp.tile([20, 128], BF16, tag="ps")
                    nc.tensor.transpose(psm[:], stage[:, mt, :], identh[:])
                    nc.scalar.copy(psiT[:, mt * 128:(mt + 1) * 128], psm[:])

                # ---------------- encoder ----------------
                xftt = tmp.tile([128, 25], F32R)
                nc.sync.dma_start(xftt[:], xft[:])
                w1c = []
                for j in range(5):
                    w1cj = tmp.tile([128, 5, 512], F32R, tag=f"w1c{j}")
                    nc.sync.dma_start(w1cj[:], w1[:, 5 * j:5 * j + 5, :])
                    w1c.append(w1cj)

                # mm1, orientation A (f32r): y1 [1, 512] accumulated over 25 K
                y1p = psp.tile([1, 512], F32, tag="ps")
                for kt in range(25):
                    nc.tensor.matmul(
                        y1p[:],
                        xftt[:, kt:kt + 1],
                        w1c[kt // 5][:, kt % 5, :],
                        start=(kt == 0),
                        stop=(kt == 24),
                    )
                h1sb = tmp.tile([1, 512], F32)
                nc.scalar.activation(h1sb[:], y1p[:], AFT.Relu)
                for mc in range(4):
                    tp1 = psp.tile([128, 1], F32, tag="ps")
                    nc.tensor.transpose(
                        tp1[:], h1sb[0:1, mc * 128:(mc + 1) * 128],
                        ident[0:1, 0:1],
                    )
                    nc.scalar.copy(h1T[:, mc:mc + 1], tp1[:])
                nc.vector.tensor_copy(h1T[:, 4:5], onesp[:])

                y2p = psp.tile([1, 128], F32, tag="ps")
                for kt in range(5):
                    nc.tensor.matmul(
                        y2p[:], h1T[:, kt:kt + 1], w2t[:, kt, :],
                        start=(kt == 0), stop=(kt == 4),
                    )
                h2sb = tmp.tile([1, 128], F32)
                nc.scalar.activation(h2sb[:], y2p[:], AFT.Relu)
                tp2 = psp.tile([128, 1], F32, tag="ps")
                nc.tensor.transpose(tp2[:], h2sb[:], ident[0:1, 0:1])
                nc.scalar.copy(h2T[:, 0:1], tp2[:])
                nc.vector.tensor_copy(h2T[:, 1:2], onesp[:])

                zp = psp.tile([1, 64], F32, tag="ps")
                for kt in range(2):
                    nc.tensor.matmul(
                        zp[:], h2T[:, kt:kt + 1], w3t[:, kt, :],
                        start=(kt == 0), stop=(kt == 1),
                    )
                zsb = tmp.tile([1, 64], F32)
                nc.scalar.activation(zsb[:], zp[:], AFT.Relu)
                tp3 = psp.tile([64, 1], F32, tag="ps")
                nc.tensor.transpose(tp3[:], zsb[:], ident[0:1, 0:1])
                nc.scalar.copy(zrelu[:], tp3[:])

                # ---------------- decoder ----------------
                nc.vector.tensor_copy(zbT[:], zrelu[:].broadcast_to([64, 64]))
                nc.vector.tensor_copy(onesb[:], onesp[:].broadcast_to([128, 64]))

                d1p = psp.tile([64, 128], F32, tag="ps")
                nc.tensor.matmul(d1p[:], zbT[:].bitcast(F32R),
                                 wd1t[:].bitcast(F32R), start=True, stop=False)
                nc.tensor.matmul(
                    d1p[:], gridtt[:].bitcast(F32R), wd1gt[:].bitcast(F32R),
                    start=False, stop=True
                )
                nc.scalar.activation(h1d[:], d1p[:], AFT.Relu)

                tr1p = psp.tile([128, 64], F32, tag="ps")
                nc.tensor.transpose(tr1p[:], h1d[:], ident[0:64, 0:64])
                nc.scalar.copy(h1dT[:], tr1p[:])

                d2p = psp.tile([64, 512], F32, tag="ps")
                nc.tensor.matmul(
                    d2p[:], h1dT[:], wd2t[:, 0, :], start=True, stop=False
                )
                nc.tensor.matmul(
                    d2p[:], onesb[:], wd2t[:, 1, :], start=False, stop=True
                )
                nc.scalar.activation(h2d[:], d2p[:], AFT.Relu)

                for kt in range(4):
                    trp = psp.tile([128, 64], F32, tag="ps")
                    nc.tensor.transpose(
                        trp[:], h2d[:, kt * 128:(kt + 1) * 128],
                        ident[0:64, 0:64],
                    )
                    nc.scalar.copy(h2dT[:, kt, :], trp[:])

                d3p = psp.tile([64, 1024], F32, tag="ps")
                for c0, w in ((0, 512), (512, 256)):
                    for kt in range(4):
                        nc.tensor.matmul(
                            d3p[:, c0:c0 + w], h2dT[:, kt, :],
                            wd3t[:, kt, c0:c0 + w],
                            start=(kt == 0), stop=False,
                        )
                    nc.tensor.matmul(
                        d3p[:, c0:c0 + w], onesb[:], wd3t[:, 4, c0:c0 + w],
                        start=False, stop=True,
                    )

                Yx = tmp.tile([64, 256], F32)
                Yy = tmp.tile([64, 256], F32)
                Yz = tmp.tile([64, 256], F32)
                nc.scalar.activation(Yx[:], d3p[:, 0:768:3], AFT.Tanh)
                nc.scalar.activation(Yy[:], d3p[:, 1:768:3], AFT.Tanh)
                nc.scalar.activation(Yz[:], d3p[:, 2:768:3], AFT.Tanh)

                # ---------------- phi (generated lift) ----------------
                n2y = tmp.tile([64, 256], F32)
                tmp2 = tmp.tile([64, 256], F32)
                nc.vector.tensor_tensor(n2y[:], Yx[:], Yx[:], op=A.mult)
                nc.vector.tensor_tensor(tmp2[:], Yy[:], Yy[:], op=A.mult)
                nc.vector.tensor_tensor(n2y[:], n2y[:], tmp2[:], op=A.add)
                nc.vector.tensor_tensor(tmp2[:], Yz[:], Yz[:], op=A.mult)
                nc.vector.tensor_tensor(n2y[:], n2y[:], tmp2[:], op=A.add)

                Lst = tmp.tile([64, 2560], BF16)
                for k, src in ((0, Yx), (1, Yy), (2, Yz), (3, n2y)):
                    nc.vector.tensor_copy(Lst[:, k * 256:(k + 1) * 256], src[:])
                nc.vector.memset(Lst[:, 4 * 256:5 * 256], 1.0)
                hk = tmp.tile([64, 256], F32)
                for k, src in ((5, Yx), (6, Yy), (7, Yz), (8, n2y)):
                    nc.vector.tensor_copy(
                        hk[:], Lst[:, (k - 5) * 256:(k - 4) * 256]
                    )
                    nc.vector.tensor_tensor(
                        Lst[:, k * 256:(k + 1) * 256], src[:], hk[:],
                        op=A.subtract,
                    )
                nc.vector.memset(Lst[:, 9 * 256:10 * 256], 0.0)

                nc.sync.dma_start(dsc[:], Lst[:])
                src3 = dsc[:].rearrange("g (k j) -> k g j", k=10)
                nc.sync.dma_start(
                    phiT[0:10, :].rearrange("k (g j) -> k g j", g=64), src3
                )
                nc.sync.dma_start(
                    phiT[10:20, :].rearrange("k (g j) -> k g j", g=64), src3
                )

            # ---------------- distance phase ----------------
            # (separate pool scope so it reuses the closed tmp pool's SBUF)
            with tc.tile_pool(name="dist", bufs=2) as distp:
              for t in range(NT):
                  bi = t % TB
                  bidx = t // TB
                  if bi == 0:
                      bb = distp.tile([128, TB, 1024], F16, tag="bb")
                  ps = psp.tile([128, 1024], F32, tag="ps")
                  lhs = phiT[:, t * 128:(t + 1) * 128]
                  nc.tensor.matmul(
                      ps[:, 0:512], lhs, psiT[:, 0:512], start=True, stop=True
                  )
                  nc.tensor.matmul(
                      ps[:, 512:1024], lhs, psiT[:, 512:1024],
                      start=True, stop=True,
                  )
                  nc.scalar.copy(bb[:, bi, :], ps[:])
                  if bi == TB - 1:
                      f1 = distp.tile([128, TB, 512], F16, tag="f1")
                      nc.vector.tensor_tensor(
                          f1[:], bb[:, :, 0:512], bb[:, :, 512:1024], op=A.min
                      )
                      f2 = distp.tile([128, TB, 256], F16, tag="f2")
                      nc.vector.tensor_tensor(
                          f2[:], f1[:, :, 0:256], f1[:, :, 256:512], op=A.min
                      )
                      f3 = distp.tile([128, TB, 128], F16, tag="f3")
                      nc.vector.tensor_tensor(
                          f3[:], f2[:, :, 0:128], f2[:, :, 128:256], op=A.min
                      )
                      f4 = distp.tile([128, TB, 64], F16, tag="f4")
                      nc.vector.tensor_tensor(
                          f4[:], f3[:, :, 0:64], f3[:, :, 64:128], op=A.min
                      )
                      nc.vector.tensor_reduce(
                          rowstore[:, bidx * TB:(bidx + 1) * TB], f4[:],
                          axis=AX.X, op=A.min,
                      )
                      nc.vector.tensor_tensor(
                          colrun[:].rearrange("p t m -> p (t m)"),
                          colrun[:].rearrange("p t m -> p (t m)"),
                          bb[:].rearrange("p t m -> p (t m)"),
                          op=A.min,
                      )

              # ---------------- epilogue ----------------
              half = TB
              while half > 2:
                  half //= 2
                  nc.vector.tensor_tensor(
                      colrun[:, 0:half, :], colrun[:, 0:half, :],
                      colrun[:, half:2 * half, :], op=A.min,
                  )
              nc.vector.tensor_tensor(
                  colrun[:, 0:1, :], colrun[:, 0:1, :], colrun[:, 1:2, :],
                  op=A.min,
              )
              nc.vector.tensor_copy(c1f[:], colrun[:, 0, :])

              for t in range(8):
                  trp2 = psp.tile([128, 128], F32, tag="ps")
                  nc.tensor.transpose(
                      trp2[:], c1f[:, t * 128:(t + 1) * 128], ident[:]
                  )
                  nc.vector.tensor_reduce(
                      colpartT[:, t:t + 1], trp2[:], axis=AX.X, op=A.min
                  )
              nc.vector.tensor_reduce(rsv[:], rowstore[:], axis=AX.X, op=A.add)

              nc.sync.dma_start(colpart[:], colpartT[:])
              nc.sync.dma_start(rowsumv[:], rsv[:])

    return nc


_NC_CACHE = {}


def _get_nc():
    if "nc" not in _NC_CACHE:
        _NC_CACHE["nc"] = _build_nc()
    return _NC_CACHE["nc"]


def _fp22(a):
    """Truncate f32 mantissa to 13 bits (FP32r) so DMA'd data is pre-rounded."""
    b = np.ascontiguousarray(a, dtype=np.float32).view(np.uint32) & np.uint32(0xFFFFFC00)
    return b.view(np.float32)


def _tiles(Wb, kt):
    """[K, N] -> [128, kt, N] partition-tiled, zero-padded."""
    K, N = Wb.shape
    pad = kt * 128 - K
    if pad:
        Wb = np.concatenate([Wb, np.zeros((pad, N), np.float32)], axis=0)
    return np.ascontiguousarray(Wb.reshape(kt, 128, N).transpose(1, 0, 2))


def prepare_in_maps(x, grid, We1, be1, We2, be2, We3, be3,
                    Wd1, bd1, Wd2, bd2, Wd3, bd3):
    f = lambda a: np.asarray(a, dtype=np.float32)
    x, grid = f(x), f(grid)
    We1, be1, We2, be2, We3, be3 = map(f, (We1, be1, We2, be2, We3, be3))
    Wd1, bd1, Wd2, bd2, Wd3, bd3 = map(f, (Wd1, bd1, Wd2, bd2, Wd3, bd3))

    w1h = _fp22(_tiles(np.vstack([We1, be1[None]]), 25))
    w2h = _fp22(_tiles(np.vstack([We2, be2[None]]), 5))
    w3h = _fp22(_tiles(np.vstack([We3, be3[None]]), 2))
    wd1h = _fp22(np.ascontiguousarray(Wd1[:64]))
    wd1gh = _fp22(np.vstack([Wd1[64:67], bd1[None]]))
    gridth = _fp22(np.vstack([grid.T, np.ones((1, G), np.float32)]))
    wd2h = _fp22(_tiles(np.vstack([Wd2, bd2[None]]), 2))
    wd3qh = [
        _fp22(_tiles(
            np.vstack([Wd3[:, 768 * q:768 * (q + 1)],
                       bd3[768 * q:768 * (q + 1)][None]]), 5
        ))
        for q in range(4)
    ]
    onespad = np.zeros((128, 1), np.float32)
    onespad[0, 0] = 1.0

    xfth = []
    s3h = []
    for b in range(B):
        xf_aug = np.zeros(3200, np.float32)
        xf_aug[:3072] = x[b].reshape(-1)
        xf_aug[3072] = 1.0
        xfth.append(_fp22(np.ascontiguousarray(xf_aug.reshape(25, 128).T)))
        # s3tl[p, mt, :] = x[b, mt*128 + p, :]
        s3h.append(np.ascontiguousarray(
            x[b].reshape(8, 128, 3).transpose(1, 0, 2)))

    in_maps = []
    for c in range(NCORES):
        b, q = c // 4, c % 4
        in_maps.append({
            "xft": xfth[b], "w1": w1h, "w2": w2h, "w3": w3h,
            "wd1": wd1h, "wd1g": wd1gh, "gridt": gridth,
            "wd2": wd2h, "wd3": wd3qh[q],
            "s3tl": s3h[b], "onespad": onespad,
        })
    return in_maps


def combine(results):
    loss = 0.0
    for c in range(NCORES):
        loss += float(results[c]["rowsumv"].astype(np.float64).sum())
    for b in range(B):
        parts = np.stack([results[c]["colpart"] for c in range(4 * b, 4 * b + 4)])
        loss += float(parts.min(axis=0).astype(np.float64).sum())
    return np.float32(loss)


def kernel(x, grid, We1, be1, We2, be2, We3, be3,
           Wd1, bd1, Wd2, bd2, Wd3, bd3, **run_kwargs):
    nc = _get_nc()
    in_maps = prepare_in_maps(x, grid, We1, be1, We2, be2, We3, be3,
                              Wd1, bd1, Wd2, bd2, Wd3, bd3)
    res = run_bass_kernel_spmd(nc, in_maps, core_ids=list(range(NCORES)),
                               **run_kwargs)
    out = combine(res.results)
    kernel.last_results = res
    return out



# revision 6
# speedup vs baseline: 1.1056x; 1.1056x over previous
"""Trainium2 Bass kernel for nn_AutoEncoder3D (chamfer-loss autoencoder).

Strategy (8 NeuronCores, SPMD with per-core data):
  core c -> batch b = c // 4, quarter q = c % 4 of generated points.
  Each core: full encoder, decoder for its quarter of the 3072 output
  columns, then fused cdist+min over the [16384, 1024] chamfer block
  using a lifted-embedding hi/lo bf16 matmul (K=13, ~1e-5 exact).

  Distance phase: n-tiles processed in PAIRS on two PE row groups
  (partitions 0-12 / 32-44) so both matmuls run concurrently.  ACT
  converts each [128, 2048] f32 PSUM pair to f16 in one copy.  DVE per
  16-tile batch: column-min tree into a [128, 1024] running min, then
  an IN-PLACE row-min fold tree on bb + one tensor_reduce into
  rowstore.  Host combines per-core row-min sums and col-min partials.
"""

import numpy as np

import concourse.bass as bass
import concourse.mybir as mybir
import concourse.tile as tile_mod
from concourse.bass_utils import run_bass_kernel_spmd
from concourse.masks import make_identity
from concourse.tile import ScopedClock, TileContext

F32 = mybir.dt.float32
F32R = mybir.dt.float32r
F16 = mybir.dt.float16
BF16 = mybir.dt.bfloat16
A = mybir.AluOpType
AFT = mybir.ActivationFunctionType
AX = mybir.AxisListType

B = 2
G = 64
M = 1024
NCORES = 8
JQ = 256          # generated points per grid cell handled per core
NLOC = G * JQ     # 16384 generated points per core
NT = NLOC // 128  # 128 n-tiles
NPAIR = NT // 2   # 64 pairs (lo half g 0-31, hi half g 32-63)
TBP = 8           # pairs per DVE batch (16 n-tiles)
NB = NPAIR // TBP  # 8 batches
KL = 13           # lift dims: [Yh3, n2yh, 1, Yl3, n2yl, Yh3, 1]


# ---------------------------------------------------------------------------
# Tile-framework patches: this walrus build allows at most ONE sync wait per
# instruction.  (a) split multi-wait instructions with preceding no-ops,
# (b) replace the context-exit drain (which carries one wait per live proc)
# with individual SP wait_ge instructions.
# ---------------------------------------------------------------------------
if not getattr(tile_mod, "_ae3d_wait_patch", False):
    tile_mod._ae3d_wait_patch = True


    _orig_commit = tile_mod.TileContext._commit_instruction

    def _commit_split(self, inst, lazy_reg_writes=True):
        si = getattr(inst, "sync_info", None)
        if si is not None and si.on_wait and len(si.on_wait) > 1:
            waits = list(si.on_wait)
            for w in waits[:-1]:
                nop = mybir.InstNoOp(
                    name=self.nc.get_next_instruction_name(),
                    sync_info=mybir.SyncInfo(on_wait=[w], on_update=[]),
                    bass_nofuse=True,
                    engine=inst.engine,
                )
                _orig_commit(self, nop, lazy_reg_writes)
            inst.sync_info = mybir.SyncInfo(
                on_wait=[waits[-1]], on_update=list(si.on_update)
            )
        return _orig_commit(self, inst, lazy_reg_writes)

    tile_mod.TileContext._commit_instruction = _commit_split

    def _patched_drain_and_barrier(self, tick_clock, wait_clock):
        gc = tick_clock.global_clock
        alloc = self.sems.allocated()
        for proc, sem in sorted(alloc.items()):
            tick = gc[proc]
            if tick > 0:
                mult = 16 if sem.name.startswith("DMA") else 1
                self.nc.sync.wait_ge(sem, tick * mult)
        self.nc.sync.drain()
        self.nc.all_engine_barrier()
        assert self.sems is not None
        popped = self.nc._tile_sem_poison_stack.pop()
        assert popped is self._sem_poison
        self.nc.clear_and_free_semaphores(list(self.sems.allocated().values()))
        self.nc.all_engine_barrier()

    tile_mod.TileContext._drain_and_barrier = _patched_drain_and_barrier


# ---------------------------------------------------------------------------
# Device program
# ---------------------------------------------------------------------------
def _build_nc():
    nc = bass.Bass()

    xft = nc.dram_tensor("xft", [128, 25], F16, kind="ExternalInput")
    w1 = nc.dram_tensor("w1", [128, 25, 512], F16, kind="ExternalInput")
    w2 = nc.dram_tensor("w2", [128, 5, 128], F32R, kind="ExternalInput")
    w3 = nc.dram_tensor("w3", [128, 2, 64], F32R, kind="ExternalInput")
    wd1 = nc.dram_tensor("wd1", [64, 128], F32R, kind="ExternalInput")
    wd1g = nc.dram_tensor("wd1g", [4, 128], F32R, kind="ExternalInput")
    gridt = nc.dram_tensor("gridt", [4, 64], F32R, kind="ExternalInput")
    wd2 = nc.dram_tensor("wd2", [128, 2, 512], F32R, kind="ExternalInput")
    wd3 = nc.dram_tensor("wd3", [128, 5, 768], F32R, kind="ExternalInput")
    s3tl = nc.dram_tensor("s3tl", [128, 8, 3], F32, kind="ExternalInput")
    onespad = nc.dram_tensor("onespad", [128, 1], F32R, kind="ExternalInput")

    colpart = nc.dram_tensor("colpart", [128, 8], F32, kind="ExternalOutput")
    rowsumv = nc.dram_tensor("rowsumv", [128, 1], F32, kind="ExternalOutput")

    # per-k bounce tensors for the phi transpose
    dscs = [nc.dram_tensor(f"dsc{k}", [64, 256], BF16) for k in range(9)]

    with TileContext(nc) as tc:
        with tc.tile_pool(name="pers", bufs=1) as pers, \
             tc.tile_pool(name="wts", bufs=1) as wts:

            # ---------------- persistent weight DMAs ----------------
            w2t = wts.tile([128, 5, 128], F32R)
            nc.sync.dma_start(w2t[:], w2[:])
            w3t = wts.tile([128, 2, 64], F32R)
            nc.sync.dma_start(w3t[:], w3[:])
            wd1t = wts.tile([64, 128], F32R)
            nc.sync.dma_start(wd1t[:], wd1[:])
            wd1gt = wts.tile([4, 128], F32R)
            nc.sync.dma_start(wd1gt[:], wd1g[:])
            gridtt = wts.tile([4, 64], F32R)
            nc.sync.dma_start(gridtt[:], gridt[:])
            wd2t = wts.tile([128, 2, 512], F32R)
            nc.sync.dma_start(wd2t[:], wd2[:])
            wd3t = wts.tile([128, 5, 768], F32R)
            nc.sync.dma_start(wd3t[:], wd3[:])
            onesp = wts.tile([128, 1], F32R)
            nc.sync.dma_start(onesp[:], onespad[:])
            ident = wts.tile([128, 128], F32)
            make_identity(nc, ident[:])
            identh = wts.tile([128, 128], BF16)
            make_identity(nc, identh[:])
            identf = wts.tile([128, 128], F16)
            make_identity(nc, identf[:])

            # psi replicated on PE row groups 0-12 and 32-44
            psiT4 = pers.tile([128, 1024], BF16)
            # phi rows 0-12 = n-tiles 0-63 (g 0-31); rows 32-44 = 64-127
            phiT2 = pers.tile([128, 8192], BF16)
            colrun = pers.tile([128, 1024], F16)
            rowstore = pers.tile([128, NT], F32)
            h1T = pers.tile([128, 5], F32R)
            h2T = pers.tile([128, 2], F32R)
            zrelu = pers.tile([64, 1], F32)
            zbT = pers.tile([64, 64], F32R)
            onesb = pers.tile([128, 64], F32R)
            h1d = pers.tile([64, 128], F32)
            h1dT = pers.tile([128, 64], F32R)
            h2d = pers.tile([64, 512], F32)
            h2dT = pers.tile([128, 4, 64], F32R)
            colpartT = pers.tile([128, 8], F32)
            rsv = pers.tile([128, 1], F32)

            with tc.tile_pool(name="tmp", bufs=1) as tmp, \
                 tc.tile_pool(name="ppre", bufs=3, space="PSUM") as psp1:
                # ---------------- PE warmup (HAM un-throttle) ----------------
                warm = psp1.tile([128, 128], F32, tag="warm", bufs=1)
                for i in range(40):
                    nc.tensor.matmul(warm[:], identh[:], identh[:],
                                     start=True, stop=True)

                # ---------------- psi (target lift) ----------------
                # stage k-layout: [m2h(3), 1, s2h, m2h(3), 1, m2l(3), s2l]
                s3t = tmp.tile([128, 8, 3], F32)
                nc.sync.dma_start(s3t[:], s3tl[:])
                stage = tmp.tile([128, 8, KL], BF16)
                sq = tmp.tile([128, 8, 3], F32)
                nc.vector.tensor_tensor(sq[:], s3t[:], s3t[:], op=A.mult)
                s2t = tmp.tile([128, 8], F32)
                nc.vector.tensor_reduce(s2t[:], sq[:], axis=AX.X, op=A.add)
                m2 = tmp.tile([128, 8, 3], F32)
                nc.vector.tensor_scalar_mul(m2[:], s3t[:], -2.0)
                s2v = s2t[:].rearrange("p (t o) -> p t o", o=1)
                nc.vector.tensor_copy(stage[:, :, 0:3], m2[:])
                nc.vector.memset(stage[:, :, 3:4], 1.0)
                nc.vector.tensor_copy(stage[:, :, 4:5], s2v)
                nc.vector.tensor_copy(stage[:, :, 5:8], stage[:, :, 0:3])
                nc.vector.memset(stage[:, :, 8:9], 1.0)
                m2hf = tmp.tile([128, 8, 3], F32)
                nc.vector.tensor_copy(m2hf[:], stage[:, :, 0:3])
                nc.vector.tensor_tensor(
                    stage[:, :, 9:12], m2[:], m2hf[:], op=A.subtract
                )
                s2hf = tmp.tile([128, 8], F32)
                nc.vector.tensor_copy(s2hf[:], stage[:, :, 4:5])
                nc.vector.tensor_tensor(
                    stage[:, :, 12:13], s2v,
                    s2hf[:].rearrange("p (t o) -> p t o", o=1), op=A.subtract,
                )
                for mt in range(8):
                    psm = psp1.tile([KL, 128], BF16, tag="ps")
                    nc.tensor.transpose(psm[:], stage[:, mt, :], identh[:])
                    nc.scalar.copy(psiT4[0:KL, mt * 128:(mt + 1) * 128], psm[:])
                # replicate psi onto PE row group 32.. (cross-partition => DMA)
                nc.sync.dma_start(psiT4[32:32 + KL, :], psiT4[0:KL, :])

                # ---------------- encoder ----------------
                xftt = tmp.tile([128, 25], F16)
                nc.sync.dma_start(xftt[:], xft[:])
                w1c = []
                for j in range(5):
                    w1cj = tmp.tile([128, 5, 512], F16, tag=f"w1c{j}")
                    nc.sync.dma_start(w1cj[:], w1[:, 5 * j:5 * j + 5, :])
                    w1c.append(w1cj)

                # mm1 (f16): y1 [1, 512] accumulated over 25 K-chunks
                y1p = psp1.tile([1, 512], F32, tag="ps")
                for kt in range(25):
                    nc.tensor.matmul(
                        y1p[:],
                        xftt[:, kt:kt + 1],
                        w1c[kt // 5][:, kt % 5, :],
                        start=(kt == 0),
                        stop=(kt == 24),
                    )
                h1sb = tmp.tile([1, 512], F32)
                nc.scalar.activation(h1sb[:], y1p[:], AFT.Relu)
                for mc in range(4):
                    tp1 = psp1.tile([128, 1], F32, tag="ps")
                    nc.tensor.transpose(
                        tp1[:], h1sb[0:1, mc * 128:(mc + 1) * 128],
                        ident[0:1, 0:1],
                    )
                    nc.scalar.copy(h1T[:, mc:mc + 1], tp1[:])
                nc.vector.tensor_copy(h1T[:, 4:5], onesp[:])

                y2p = psp1.tile([1, 128], F32, tag="ps")
                for kt in range(5):
                    nc.tensor.matmul(
                        y2p[:], h1T[:, kt:kt + 1], w2t[:, kt, :],
                        start=(kt == 0), stop=(kt == 4),
                    )
                h2sb = tmp.tile([1, 128], F32)
                nc.scalar.activation(h2sb[:], y2p[:], AFT.Relu)
                tp2 = psp1.tile([128, 1], F32, tag="ps")
                nc.tensor.transpose(tp2[:], h2sb[:], ident[0:1, 0:1])
                nc.scalar.copy(h2T[:, 0:1], tp2[:])
                nc.vector.tensor_copy(h2T[:, 1:2], onesp[:])

                zp = psp1.tile([1, 64], F32, tag="ps")
                for kt in range(2):
                    nc.tensor.matmul(
                        zp[:], h2T[:, kt:kt + 1], w3t[:, kt, :],
                        start=(kt == 0), stop=(kt == 1),
                    )
                zsb = tmp.tile([1, 64], F32)
                nc.scalar.activation(zsb[:], zp[:], AFT.Relu)
                tp3 = psp1.tile([64, 1], F32, tag="ps")
                nc.tensor.transpose(tp3[:], zsb[:], ident[0:1, 0:1])
                nc.scalar.copy(zrelu[:], tp3[:])

                # ---------------- decoder ----------------
                nc.vector.tensor_copy(zbT[:], zrelu[:].broadcast_to([64, 64]))
                nc.vector.tensor_copy(onesb[:], onesp[:].broadcast_to([128, 64]))

                d1p = psp1.tile([64, 128], F32, tag="ps")
                nc.tensor.matmul(d1p[:], zbT[:].bitcast(F32R),
                                 wd1t[:].bitcast(F32R), start=True, stop=False)
                nc.tensor.matmul(
                    d1p[:], gridtt[:].bitcast(F32R), wd1gt[:].bitcast(F32R),
                    start=False, stop=True
                )
                nc.scalar.activation(h1d[:], d1p[:], AFT.Relu)

                tr1p = psp1.tile([128, 64], F32, tag="ps")
                nc.tensor.transpose(tr1p[:], h1d[:], ident[0:64, 0:64])
                nc.scalar.copy(h1dT[:], tr1p[:])

                d2p = psp1.tile([64, 512], F32, tag="ps")
                nc.tensor.matmul(
                    d2p[:], h1dT[:], wd2t[:, 0, :], start=True, stop=False
                )
                nc.tensor.matmul(
                    d2p[:], onesb[:], wd2t[:, 1, :], start=False, stop=True
                )
                nc.scalar.activation(h2d[:], d2p[:], AFT.Relu)

                for kt in range(4):
                    trp = psp1.tile([128, 64], F32, tag="ps")
                    nc.tensor.transpose(
                        trp[:], h2d[:, kt * 128:(kt + 1) * 128],
                        ident[0:64, 0:64],
                    )
                    nc.scalar.copy(h2dT[:, kt, :], trp[:])

                d3p = psp1.tile([64, 1024], F32, tag="ps")
                for c0, w in ((0, 512), (512, 256)):
                    for kt in range(4):
                        nc.tensor.matmul(
                            d3p[:, c0:c0 + w], h2dT[:, kt, :],
                            wd3t[:, kt, c0:c0 + w],
                            start=(kt == 0), stop=False,
                        )
                    nc.tensor.matmul(
                        d3p[:, c0:c0 + w], onesb[:], wd3t[:, 4, c0:c0 + w],
                        start=False, stop=True,
                    )

                Yx = tmp.tile([64, 256], F32)
                Yy = tmp.tile([64, 256], F32)
                Yz = tmp.tile([64, 256], F32)
                nc.scalar.activation(Yx[:], d3p[:, 0:768:3], AFT.Tanh)
                nc.scalar.activation(Yy[:], d3p[:, 1:768:3], AFT.Tanh)
                nc.scalar.activation(Yz[:], d3p[:, 2:768:3], AFT.Tanh)

                # ---------------- phi (generated lift) ----------------
                # 9 unique rows: [Yxh, Yyh, Yzh, n2yh, 1, Yxl, Yyl, Yzl, n2yl]
                n2y = tmp.tile([64, 256], F32)
                tmp2 = tmp.tile([64, 256], F32)
                nc.vector.tensor_tensor(n2y[:], Yx[:], Yx[:], op=A.mult)
                nc.vector.tensor_tensor(tmp2[:], Yy[:], Yy[:], op=A.mult)
                nc.vector.tensor_tensor(n2y[:], n2y[:], tmp2[:], op=A.add)
                nc.vector.tensor_tensor(tmp2[:], Yz[:], Yz[:], op=A.mult)
                nc.vector.tensor_tensor(n2y[:], n2y[:], tmp2[:], op=A.add)

                Lk = []
                for k, src in ((0, Yx), (1, Yy), (2, Yz), (3, n2y)):
                    Lt = tmp.tile([64, 256], BF16, name=f"L{k}")
                    nc.vector.tensor_copy(Lt[:], src[:])
                    nc.sync.dma_start(dscs[k][:], Lt[:])
                    Lk.append(Lt)
                L4 = tmp.tile([64, 256], BF16)
                nc.vector.memset(L4[:], 1.0)
                nc.sync.dma_start(dscs[4][:], L4[:])
                Lk.append(L4)
                for k, src in ((5, Yx), (6, Yy), (7, Yz), (8, n2y)):
                    hk = tmp.tile([64, 256], F32, name=f"h{k}")
                    nc.vector.tensor_copy(hk[:], Lk[k - 5][:])
                    Lt = tmp.tile([64, 256], BF16, name=f"L{k}")
                    nc.vector.tensor_tensor(Lt[:], src[:], hk[:], op=A.subtract)
                    nc.sync.dma_start(dscs[k][:], Lt[:])
                    Lk.append(Lt)

                # phi row r sources dsc[PHSRC[r]]
                PHSRC = [0, 1, 2, 3, 4, 5, 6, 7, 8, 0, 1, 2, 4]
                for r in range(KL):
                    sk = dscs[PHSRC[r]]
                    nc.sync.dma_start(
                        phiT2[r:r + 1, :].rearrange("k (g j) -> k g j", g=32),
                        sk[0:32, :].rearrange("(o g) j -> o g j", o=1),
                    )
                    nc.sync.dma_start(
                        phiT2[32 + r:33 + r, :].rearrange(
                            "k (g j) -> k g j", g=32),
                        sk[32:64, :].rearrange("(o g) j -> o g j", o=1),
                    )

            # ---------------- distance phase ----------------
            # (separate pool scope so it reuses the closed tmp pool's SBUF)
            with tc.tile_pool(name="dist", bufs=2) as distp, \
                 tc.tile_pool(name="pdist", bufs=2, space="PSUM") as psp2:
              for p in range(NPAIR):
                  bi = p % TBP
                  bidx = p // TBP
                  if bi == 0:
                      bb = distp.tile([128, TBP, 2, 1024], F16, tag="bb")
                  ps = psp2.tile([128, 2, 1024], F32, tag="dps")
                  lhs_lo = phiT2[0:KL, p * 128:(p + 1) * 128]
                  lhs_hi = phiT2[32:32 + KL, p * 128:(p + 1) * 128]
                  nc.tensor.matmul(
                      ps[:, 0, 0:512], lhs_lo, psiT4[0:KL, 0:512],
                      start=True, stop=True,
                  )
                  nc.tensor.matmul(
                      ps[:, 1, 0:512], lhs_hi, psiT4[32:32 + KL, 0:512],
                      start=True, stop=True,
                  )
                  nc.tensor.matmul(
                      ps[:, 0, 512:1024], lhs_lo, psiT4[0:KL, 512:1024],
                      start=True, stop=True,
                  )
                  nc.tensor.matmul(
                      ps[:, 1, 512:1024], lhs_hi, psiT4[32:32 + KL, 512:1024],
                      start=True, stop=True,
                  )
                  nc.scalar.copy(
                      bb[:, bi, :, :].rearrange("p a b -> p (a b)"),
                      ps[:].rearrange("p a b -> p (a b)"),
                  )

                  if bi == TBP - 1:
                      bbt = bb[:].rearrange("p j h m -> p (j h) m")  # 16 tiles
                      # ---- column path: fold 16 tiles -> colrun [128,1024]
                      c1 = distp.tile([128, 8, 1024], F16, tag="c1")
                      nc.vector.tensor_tensor(
                          c1[:], bbt[:, 0:8, :], bbt[:, 8:16, :], op=A.min)
                      nc.vector.tensor_tensor(
                          c1[:, 0:4, :], c1[:, 0:4, :], c1[:, 4:8, :],
                          op=A.min)
                      nc.vector.tensor_tensor(
                          c1[:, 0:2, :], c1[:, 0:2, :], c1[:, 2:4, :],
                          op=A.min)
                      nc.vector.tensor_tensor(
                          c1[:, 0, :], c1[:, 0, :], c1[:, 1, :], op=A.min)
                      if bidx == 0:
                          nc.vector.tensor_copy(colrun[:], c1[:, 0, :])
                      else:
                          nc.vector.tensor_tensor(
                              colrun[:], colrun[:], c1[:, 0, :], op=A.min)
                      # ---- row path: in-place fold tree on bb
                      w = 512
                      while w >= 8:
                          nc.vector.tensor_tensor(
                              bbt[:, :, 0:w], bbt[:, :, 0:w], bbt[:, :, w:2 * w],
                              op=A.min)
                          w //= 2
                      nc.vector.tensor_reduce(
                          rowstore[:, bidx * 16:(bidx + 1) * 16],
                          bbt[:, :, 0:8], axis=AX.X, op=A.min,
                      )

              # ---------------- epilogue ----------------
              for t in range(8):
                  trp2 = psp2.tile([128, 128], F16, tag="dps")
                  nc.tensor.transpose(
                      trp2[:], colrun[:, t * 128:(t + 1) * 128], identf[:]
                  )
                  nc.vector.tensor_reduce(
                      colpartT[:, t:t + 1], trp2[:], axis=AX.X, op=A.min
                  )
              nc.vector.tensor_reduce(rsv[:], rowstore[:], axis=AX.X, op=A.add)

              nc.sync.dma_start(colpart[:], colpartT[:])
              nc.sync.dma_start(rowsumv[:], rsv[:])

    return nc


_NC_CACHE = {}


def _get_nc():
    if "nc" not in _NC_CACHE:
        _NC_CACHE["nc"] = _build_nc()
    return _NC_CACHE["nc"]


def _fp22(a):
    """Truncate f32 mantissa to 13 bits (FP32r) so DMA'd data is pre-rounded."""
    b = np.ascontiguousarray(a, dtype=np.float32).view(np.uint32) & np.uint32(0xFFFFFC00)
    return b.view(np.float32)


def _tiles(Wb, kt):
    """[K, N] -> [128, kt, N] partition-tiled, zero-padded."""
    K, N = Wb.shape
    pad = kt * 128 - K
    if pad:
        Wb = np.concatenate([Wb, np.zeros((pad, N), np.float32)], axis=0)
    return np.ascontiguousarray(Wb.reshape(kt, 128, N).transpose(1, 0, 2))


def prepare_in_maps(x, grid, We1, be1, We2, be2, We3, be3,
                    Wd1, bd1, Wd2, bd2, Wd3, bd3):
    f = lambda a: np.asarray(a, dtype=np.float32)
    x, grid = f(x), f(grid)
    We1, be1, We2, be2, We3, be3 = map(f, (We1, be1, We2, be2, We3, be3))
    Wd1, bd1, Wd2, bd2, Wd3, bd3 = map(f, (Wd1, bd1, Wd2, bd2, Wd3, bd3))

    w1h = _tiles(np.vstack([We1, be1[None]]), 25).astype(np.float16)
    w2h = _fp22(_tiles(np.vstack([We2, be2[None]]), 5))
    w3h = _fp22(_tiles(np.vstack([We3, be3[None]]), 2))
    wd1h = _fp22(np.ascontiguousarray(Wd1[:64]))
    wd1gh = _fp22(np.vstack([Wd1[64:67], bd1[None]]))
    gridth = _fp22(np.vstack([grid.T, np.ones((1, G), np.float32)]))
    wd2h = _fp22(_tiles(np.vstack([Wd2, bd2[None]]), 2))
    wd3qh = [
        _fp22(_tiles(
            np.vstack([Wd3[:, 768 * q:768 * (q + 1)],
                       bd3[768 * q:768 * (q + 1)][None]]), 5
        ))
        for q in range(4)
    ]
    onespad = np.zeros((128, 1), np.float32)
    onespad[0, 0] = 1.0

    xfth = []
    s3h = []
    for b in range(B):
        xf_aug = np.zeros(3200, np.float32)
        xf_aug[:3072] = x[b].reshape(-1)
        xf_aug[3072] = 1.0
        xfth.append(np.ascontiguousarray(
            xf_aug.reshape(25, 128).T).astype(np.float16))
        # s3tl[p, mt, :] = x[b, mt*128 + p, :]
        s3h.append(np.ascontiguousarray(
            x[b].reshape(8, 128, 3).transpose(1, 0, 2)))

    in_maps = []
    for c in range(NCORES):
        b, q = c // 4, c % 4
        in_maps.append({
            "xft": xfth[b], "w1": w1h, "w2": w2h, "w3": w3h,
            "wd1": wd1h, "wd1g": wd1gh, "gridt": gridth,
            "wd2": wd2h, "wd3": wd3qh[q],
            "s3tl": s3h[b], "onespad": onespad,
        })
    return in_maps


def combine(results):
    loss = 0.0
    for c in range(NCORES):
        loss += float(results[c]["rowsumv"].astype(np.float64).sum())
    for b in range(B):
        parts = np.stack([results[c]["colpart"] for c in range(4 * b, 4 * b + 4)])
        loss += float(parts.min(axis=0).astype(np.float64).sum())
    return np.float32(loss)


def kernel(x, grid, We1, be1, We2, be2, We3, be3,
           Wd1, bd1, Wd2, bd2, Wd3, bd3, **run_kwargs):
    nc = _get_nc()
    in_maps = prepare_in_maps(x, grid, We1, be1, We2, be2, We3, be3,
                              Wd1, bd1, Wd2, bd2, Wd3, bd3)
    res = run_bass_kernel_spmd(nc, in_maps, core_ids=list(range(NCORES)),
                               **run_kwargs)
    out = combine(res.results)
    kernel.last_results = res
    return out
